# revision 25
# baseline (speedup 1.0000x reference)
"""Trainium2 Bass kernel for nn_ContextQueryAttentionLayer.

Math: with B,N,M,D = 32,1024,256,128 the reference's gather index collapses:
  idx[i,j] = (i*M + j) % N = 256*(i%4) + j          (since M=256, N=1024)
so the similarity matrix S (b,n,m) has only 4 distinct rows per batch,
S[b,i,:] = t[b, i%4, :] with t (4,256):
  t[r,j] = q_j.w_q + c_{256r+j}.w_c + sum_d q_{j,d} w_m_d c_{256r+j,d}
Both softmaxes, c2q, sm (a 4x4 matrix per batch) and q2c collapse to
rank-4-per-batch quantities, leaving a DMA-bound kernel:
  out[b,n] = [ctx_n, C2Q[n%4], ctx_n*C2Q[n%4], ctx_n*Q2C[n%4]]

v4: per-batch pipeline with the softmax in the untransposed [128, 8]
domain (t_sb[p, 2r+h] = t[r, 128h+p]; j%128 IS the partition index so
row sums / SM4 / C2Q contract directly over partitions on the PE).
Query prep is batched once (it only needs qry+consts). The q.w_q term
rides an augmented 129th column (ctx col 128 = 1, qwc col 128 = sq) so
one fused mul+reduce pass produces t directly. Normalizations ride the
PSUM->SBUF copies as per-partition activation scales; q2c contracts ctx
on the PE via 8 accumulating matmuls with M[p,r] = SM4raw[p%4,r].
Queues: sync = ctx loads + product streams; scalar = consts, qry,
raw-ctx stream, c2q-broadcast stream.
"""

import numpy as np

B, N, M, D = 32, 1024, 256, 128
NCORES = 8
BPC = B // NCORES  # batches per core

_prog = None

# packed constant layout: name -> (partitions, col_start, col_len)
_CST_COLS = {
    "wmb": (128, 0, 128),
    "wcb": (128, 128, 128),
    "wqb": (128, 256, 128),
    "b4": (4, 384, 128),
    "pairsel": (8, 512, 4),
    "ones1": (128, 516, 1),
    "rsel": (128, 517, 4),
}
_CST_W = 524


def _build_program():
    import concourse.bacc as bacc
    import concourse.mybir as mybir
    from concourse.tile import TileContext

    fp32 = mybir.dt.float32
    nc = bacc.Bacc("TRN2", target_bir_lowering=False, name="cqattn")

    ctx_d = nc.dram_tensor("ctx", [BPC, N, D], fp32, kind="ExternalInput")
    qry_d = nc.dram_tensor("qry", [BPC, M, D], fp32, kind="ExternalInput")
    cstp_d = nc.dram_tensor("cstp", [128, _CST_W], fp32, kind="ExternalInput")
    out_d = nc.dram_tensor("out", [BPC, N, 4 * D], fp32, kind="ExternalOutput")

    Exp = mybir.ActivationFunctionType.Exp
    Copy = mybir.ActivationFunctionType.Copy
    add = mybir.AluOpType.add
    X = mybir.AxisListType.X

    with TileContext(nc) as tc:
        with (
            tc.tile_pool(name="consts", bufs=1) as consts,
            tc.tile_pool(name="io", bufs=1) as io,
            tc.tile_pool(name="work", bufs=2) as work,
            tc.tile_pool(name="small", bufs=2) as small,
            tc.tile_pool(name="outp", bufs=2) as outp,
            tc.tile_pool(name="ps_sm", bufs=2, space="PSUM") as ps_sm,
            tc.tile_pool(name="ps_mm", bufs=2, space="PSUM") as ps_mm,
            tc.tile_pool(name="ps_rep", bufs=2, space="PSUM") as ps_rep,
        ):
            # ---- consts then qry on the scalar (ACT) queue, first thing
            cstp = consts.tile([128, _CST_W], fp32, tag="cstp", name="cstp")
            nc.scalar.dma_start(out=cstp, in_=cstp_d[...])
            cst = {
                n: cstp[:p, c0 : c0 + cl] for n, (p, c0, cl) in _CST_COLS.items()
            }
            qry_mega = io.tile([128, BPC, 2, 128], fp32, tag="qry", name="qry_mega")
            nc.scalar.dma_start(
                out=qry_mega,
                in_=qry_d.rearrange("b (h p) d -> p b h d", p=128),
            )

            # ---- ctx loads back-to-back on the sync queue
            ctx_mega = io.tile([128, BPC, 8, 128], fp32, tag="ctx", name="ctx_mega")
            for b in range(BPC):
                nc.sync.dma_start(
                    out=ctx_mega[:, b],
                    in_=ctx_d[b].rearrange("(k p) d -> p k d", p=128),
                )

            # ---- batched query prep: qwc = qry*wm + wc, sq = qry . wq
            qwc = work.tile([128, BPC, 2, 128], fp32, tag="qwc", name="qwc")
            nc.vector.tensor_mul(
                qwc,
                qry_mega,
                cst["wmb"]
                .rearrange("p (u v d) -> p u v d", u=1, v=1)
                .to_broadcast([128, BPC, 2, 128]),
            )
            nc.vector.tensor_add(
                qwc,
                qwc,
                cst["wcb"]
                .rearrange("p (u v d) -> p u v d", u=1, v=1)
                .to_broadcast([128, BPC, 2, 128]),
            )
            sq_tmp = work.tile([128, BPC, 2, 128], fp32, tag="sq_tmp")
            nc.vector.tensor_mul(
                sq_tmp,
                qry_mega,
                cst["wqb"]
                .rearrange("p (u v d) -> p u v d", u=1, v=1)
                .to_broadcast([128, BPC, 2, 128]),
            )
            sq_col = small.tile([128, BPC, 2], fp32, tag="sq_col")
            nc.vector.tensor_reduce(out=sq_col, in_=sq_tmp, axis=X, op=add)

            prev_recu_inst = None
            for b in range(BPC):
                ctx_b = ctx_mega[:, b]
                qry_b = qry_mega[:, b]

                # output stream a: raw context columns (scalar queue)
                nc.scalar.dma_start(
                    out=out_d[b][:, 0:128].rearrange("(k p) c -> p k c", p=128),
                    in_=ctx_b,
                )

                # ---- t columns: t_sb[p, 2r+h] = t[r, 128h+p]
                g_tmp = work.tile([128, 2, 4, 128], fp32, tag="g_tmp")
                g_mul_inst = nc.vector.tensor_mul(
                    g_tmp,
                    ctx_b.rearrange("p (r h) d -> p h r d", h=2),
                    qwc[:, b]
                    .rearrange("p h (u d) -> p h u d", u=1)
                    .to_broadcast([128, 2, 4, 128]),
                )
                if prev_recu_inst is not None:
                    from concourse.tile_rust import add_dep_helper

                    add_dep_helper(
                        g_mul_inst.ins,
                        prev_recu_inst.ins,
                        sync=False,
                        reason="keep DVE chain smalls ahead of next batch bulk",
                    )
                t_sb = small.tile([128, 8], fp32, tag="t_sb")
                nc.vector.tensor_reduce(
                    out=t_sb[:, :].rearrange("p (r h) -> p h r", h=2),
                    in_=g_tmp,
                    axis=X,
                    op=add,
                )
                nc.vector.tensor_add(
                    t_sb[:, :].rearrange("p (r h) -> p r h", h=2),
                    t_sb[:, :].rearrange("p (r h) -> p r h", h=2),
                    sq_col[:, b, :]
                    .rearrange("p (u h) -> p u h", u=1)
                    .to_broadcast([128, 4, 2]),
                )

                # ---- CS tree (independent of softmax): csum[p,d]=sum_k ctx
                tmp4 = work.tile([128, 4, 128], fp32, tag="tmp4")
                nc.vector.tensor_add(
                    tmp4, ctx_b[:, 0:4, :], ctx_b[:, 4:8, :]
                )
                tmp2 = work.tile([128, 2, 128], fp32, tag="tmp2")
                nc.gpsimd.tensor_add(tmp2, tmp4[:, 0:2, :], tmp4[:, 2:4, :])
                csum = work.tile([128, 128], fp32, tag="csum")
                nc.gpsimd.tensor_add(csum, tmp2[:, 0, :], tmp2[:, 1, :])
                cs4_ps = ps_mm.tile([4, 128], fp32, tag="mm")
                nc.tensor.matmul(cs4_ps, cst["rsel"], csum, start=True, stop=True)
                cs4 = small.tile([4, 128], fp32, tag="cs4")
                nc.scalar.activation(
                    out=cs4, in_=cs4_ps, func=Copy, scale=1.0 / 256.0
                )

                # ---- softmax pieces, untransposed domain (|t| < ~8)
                e_sb = small.tile([128, 8], fp32, tag="e_sb")
                nc.scalar.activation(out=e_sb, in_=t_sb, func=Exp)
                ev = e_sb[:, :].rearrange("p (r h) -> p r h", r=4)
                # Z_r = sum_j e[r,j]: two accumulating column-sum matmuls
                pairs_ps = ps_sm.tile([4, 1], fp32, tag="sm")
                for h in range(2):
                    nc.tensor.matmul(
                        pairs_ps, ev[:, :, h], cst["ones1"],
                        start=(h == 0), stop=(h == 1),
                    )
                rec4 = small.tile([4, 1], fp32, tag="rec4")
                nc.vector.reciprocal(out=rec4, in_=pairs_ps)
                # U[j] = sum_r e[r,j] lives per-partition: strided reduce
                u_sb = small.tile([128, 2], fp32, tag="u_sb")
                nc.vector.tensor_reduce(
                    out=u_sb,
                    in_=e_sb[:, :].rearrange("p (r h) -> p h r", h=2),
                    axis=X,
                    op=add,
                )
                recu = small.tile([128, 2], fp32, tag="recu")
                prev_recu_inst = nc.vector.reciprocal(out=recu, in_=u_sb)
                sqn = small.tile([128, 8], fp32, tag="sqn")
                nc.gpsimd.tensor_mul(
                    sqn[:, :].rearrange("p (r h) -> p r h", r=4),
                    ev,
                    recu[:, :]
                    .rearrange("p (u h) -> p u h", u=1)
                    .to_broadcast([128, 4, 2]),
                )
                sqnv = sqn[:, :].rearrange("p (r h) -> p r h", r=4)

                # ---- SM4raw[r',r] = sum_j sqn[r',j] e[r,j]
                sm4t_ps = ps_mm.tile([4, 4], fp32, tag="mm")
                for h in range(2):
                    nc.tensor.matmul(
                        sm4t_ps, sqnv[:, :, h], ev[:, :, h],
                        start=(h == 0), stop=(h == 1),
                    )
                sm4t = small.tile([4, 4], fp32, tag="sm4t")
                nc.scalar.copy(out=sm4t, in_=sm4t_ps)

                # ---- C2Q[r,d] = (1/Z_r) sum_j e[r,j] qry[j,d]
                c2q_ps = ps_mm.tile([4, 128], fp32, tag="mm")
                for h in range(2):
                    nc.tensor.matmul(
                        c2q_ps, ev[:, :, h], qry_b[:, h, :],
                        start=(h == 0), stop=(h == 1),
                    )
                c2q = small.tile([4, 128], fp32, tag="c2q")
                nc.scalar.activation(out=c2q, in_=c2q_ps, func=Copy, scale=rec4)
                repc_ps = ps_rep.tile([128, 128], fp32, tag="rep")
                nc.tensor.matmul(repc_ps, cst["b4"], c2q, start=True, stop=True)
                repc = small.tile([128, 128], fp32, tag="repc")
                nc.scalar.copy(out=repc, in_=repc_ps)

                # output stream b: broadcast C2Q columns (scalar queue)
                nc.scalar.dma_start(
                    out=out_d[b][:, 128:256].rearrange("(k p) c -> p k c", p=128),
                    in_=repc[:, :]
                    .rearrange("p (u d) -> p u d", u=1)
                    .to_broadcast([128, 8, 128]),
                )

                # ---- Q2C[r,d] = (1/(256 Z_r)) sum_r' SM4raw[r',r] CS[r',d]
                q2c_ps = ps_mm.tile([4, 128], fp32, tag="mm")
                nc.tensor.matmul(q2c_ps, sm4t, cs4, start=True, stop=True)
                q2c = small.tile([4, 128], fp32, tag="q2c")
                nc.scalar.activation(out=q2c, in_=q2c_ps, func=Copy, scale=rec4)
                repq_ps = ps_rep.tile([128, 128], fp32, tag="rep")
                nc.tensor.matmul(repq_ps, cst["b4"], q2c, start=True, stop=True)
                repq = small.tile([128, 128], fp32, tag="repq")
                nc.scalar.copy(out=repq, in_=repq_ps)

                # ---- product streams (sync queue), c first
                out_c = outp.tile([128, 8, 128], fp32, tag="out_c")
                nc.vector.tensor_mul(
                    out_c,
                    ctx_b,
                    repc[:, :]
                    .rearrange("p (u d) -> p u d", u=1)
                    .to_broadcast([128, 8, 128]),
                )
                nc.sync.dma_start(
                    out=out_d[b][:, 256:384].rearrange("(k p) c -> p k c", p=128),
                    in_=out_c,
                )
                out_dd = outp.tile([128, 8, 128], fp32, tag="out_d")
                nc.gpsimd.tensor_mul(
                    out_dd,
                    ctx_b,
                    repq[:, :]
                    .rearrange("p (u d) -> p u d", u=1)
                    .to_broadcast([128, 8, 128]),
                )
                nc.sync.dma_start(
                    out=out_d[b][:, 384:512].rearrange("(k p) c -> p k c", p=128),
                    in_=out_dd,
                )
    nc.compile()
    return nc


def _get_program():
    global _prog
    if _prog is None:
        _prog = _build_program()
    return _prog


def _make_const_inputs(w):
    w = np.ascontiguousarray(w, dtype=np.float32)
    w_q, w_c, w_m = w[:D, 0], w[D : 2 * D, 0], w[2 * D :, 0]
    p = np.arange(128)
    pairsel = (np.arange(8)[:, None] // 2 == np.arange(4)[None, :]).astype(
        np.float32
    )
    vals = {
        "wmb": np.broadcast_to(w_m[None, :], (128, 128)),
        "wcb": np.broadcast_to(w_c[None, :], (128, 128)),
        "wqb": np.broadcast_to(w_q[None, :], (128, 128)),
        "pairsel": pairsel,
        "b4": (np.arange(4)[:, None] == p[None, :] % 4).astype(np.float32),
        "ones1": np.ones((128, 1), np.float32),
        "rsel": (p[:, None] % 4 == np.arange(4)[None, :]).astype(np.float32),
    }
    packed = np.zeros((128, _CST_W), dtype=np.float32)
    for n, (parts, c0, cl) in _CST_COLS.items():
        packed[:parts, c0 : c0 + cl] = vals[n]
    return {"cstp": packed}


def _run(context, query, w, trace=False):
    from concourse.bass_utils import run_bass_kernel_spmd

    nc = _get_program()
    context = np.ascontiguousarray(context, dtype=np.float32)
    query = np.ascontiguousarray(query, dtype=np.float32)
    consts = _make_const_inputs(w)

    in_maps = []
    for c in range(NCORES):
        m = {
            "ctx": context[c * BPC : (c + 1) * BPC],
            "qry": query[c * BPC : (c + 1) * BPC],
        }
        m.update(consts)
        in_maps.append(m)

    res = run_bass_kernel_spmd(
        nc, in_maps, core_ids=list(range(NCORES)), trace=trace
    )
    out = np.concatenate([res.results[c]["out"] for c in range(NCORES)], axis=0)
    return out, res


def kernel(context, query, c_mask, q_mask, w):
    out, _ = _run(context, query, w, trace=False)
    return out


# revision 28
# speedup vs baseline: 1.0588x; 1.0588x over previous
"""Trainium2 Bass kernel for nn_ContextQueryAttentionLayer.

Math: with B,N,M,D = 32,1024,256,128 the reference's gather index collapses:
  idx[i,j] = (i*M + j) % N = 256*(i%4) + j          (since M=256, N=1024)
so the similarity matrix S (b,n,m) has only 4 distinct rows per batch,
S[b,i,:] = t[b, i%4, :] with t (4,256):
  t[r,j] = q_j.w_q + c_{256r+j}.w_c + sum_d q_{j,d} w_m_d c_{256r+j,d}
Both softmaxes, c2q, sm (a 4x4 matrix per batch) and q2c collapse to
rank-4-per-batch quantities, leaving a DMA-bound kernel:
  out[b,n] = [ctx_n, C2Q[n%4], ctx_n*C2Q[n%4], ctx_n*Q2C[n%4]]

v4: per-batch pipeline with the softmax in the untransposed [128, 8]
domain (t_sb[p, 2r+h] = t[r, 128h+p]; j%128 IS the partition index so
row sums / SM4 / C2Q contract directly over partitions on the PE).
Query prep is batched once (it only needs qry+consts). The q.w_q term
rides an augmented 129th column (ctx col 128 = 1, qwc col 128 = sq) so
one fused mul+reduce pass produces t directly. Normalizations ride the
PSUM->SBUF copies as per-partition activation scales; q2c contracts ctx
on the PE via 8 accumulating matmuls with M[p,r] = SM4raw[p%4,r].
Queues: sync = ctx loads + product streams; scalar = consts, qry,
raw-ctx stream, c2q-broadcast stream.
"""

import numpy as np

B, N, M, D = 32, 1024, 256, 128
NCORES = 8
BPC = B // NCORES  # batches per core
DA = D + 1  # augmented depth: col 128 carries the s_q term

_prog = None

# packed constant layout: name -> (partitions, col_start, col_len)
_CST_COLS = {
    "wmb": (128, 0, 128),
    "wcb": (128, 128, 128),
    "wqb": (128, 256, 128),
    "b4": (4, 384, 128),
    "pairsel": (8, 512, 4),
    "ones1": (128, 516, 1),
    "rsel": (128, 517, 4),
}
_CST_W = 524


def _build_program():
    import concourse.bacc as bacc
    import concourse.mybir as mybir
    from concourse.tile import TileContext

    fp32 = mybir.dt.float32
    nc = bacc.Bacc("TRN2", target_bir_lowering=False, name="cqattn")

    ctx_d = nc.dram_tensor("ctx", [BPC, N, D], fp32, kind="ExternalInput")
    qry_d = nc.dram_tensor("qry", [BPC, M, D], fp32, kind="ExternalInput")
    cstp_d = nc.dram_tensor("cstp", [128, _CST_W], fp32, kind="ExternalInput")
    out_d = nc.dram_tensor("out", [BPC, N, 4 * D], fp32, kind="ExternalOutput")

    Exp = mybir.ActivationFunctionType.Exp
    Copy = mybir.ActivationFunctionType.Copy
    add = mybir.AluOpType.add
    X = mybir.AxisListType.X

    with TileContext(nc) as tc:
        with (
            tc.tile_pool(name="consts", bufs=1) as consts,
            tc.tile_pool(name="io", bufs=1) as io,
            tc.tile_pool(name="work", bufs=2) as work,
            tc.tile_pool(name="small", bufs=2) as small,
            tc.tile_pool(name="outp", bufs=2) as outp,
            tc.tile_pool(name="ps_sm", bufs=2, space="PSUM") as ps_sm,
            tc.tile_pool(name="ps_mm", bufs=2, space="PSUM") as ps_mm,
            tc.tile_pool(name="ps_rep", bufs=2, space="PSUM") as ps_rep,
        ):
            # ---- consts then qry on the scalar (ACT) queue, first thing
            cstp = consts.tile([128, _CST_W], fp32, tag="cstp", name="cstp")
            nc.scalar.dma_start(out=cstp, in_=cstp_d[...])
            cst = {
                n: cstp[:p, c0 : c0 + cl] for n, (p, c0, cl) in _CST_COLS.items()
            }
            qry_mega = io.tile([128, BPC, 2, 128], fp32, tag="qry", name="qry_mega")
            nc.scalar.dma_start(
                out=qry_mega,
                in_=qry_d.rearrange("b (h p) d -> p b h d", p=128),
            )

            # ---- ctx loads back-to-back on the sync queue (col 128 = 1.0)
            ctx_mega = io.tile([128, BPC, 8, DA], fp32, tag="ctx", name="ctx_mega")
            nc.vector.memset(ctx_mega[:, :, :, 128:129], 1.0)
            for b in range(BPC):
                nc.sync.dma_start(
                    out=ctx_mega[:, b, :, 0:128],
                    in_=ctx_d[b].rearrange("(k p) d -> p k d", p=128),
                )

            for b in range(BPC):
                ctx_b = ctx_mega[:, b]
                qry_b = qry_mega[:, b]

                # ---- per-batch query prep: qwc_aug[p,h,:128] = qry*wm + wc,
                #      qwc_aug[p,h,128] = sq = qry . wq
                qwc_aug = work.tile([128, 2, DA], fp32, tag="qwc")
                qa = qwc_aug[:, :, 0:128]
                nc.vector.tensor_mul(
                    qa,
                    qry_b,
                    cst["wmb"]
                    .rearrange("p (u d) -> p u d", u=1)
                    .to_broadcast([128, 2, 128]),
                )
                nc.vector.tensor_add(
                    qa,
                    qa,
                    cst["wcb"]
                    .rearrange("p (u d) -> p u d", u=1)
                    .to_broadcast([128, 2, 128]),
                )
                sq_tmp = work.tile([128, 2, 128], fp32, tag="sq_tmp")
                nc.vector.tensor_mul(
                    sq_tmp,
                    qry_b,
                    cst["wqb"]
                    .rearrange("p (u d) -> p u d", u=1)
                    .to_broadcast([128, 2, 128]),
                )
                nc.vector.tensor_reduce(
                    out=qwc_aug[:, :, 128:129].rearrange("p h o -> p (h o)"),
                    in_=sq_tmp,
                    axis=X,
                    op=add,
                )

                # output stream a: raw context columns (scalar queue)
                nc.scalar.dma_start(
                    out=out_d[b][:, 0:128].rearrange("(k p) c -> p k c", p=128),
                    in_=ctx_b[:, :, 0:128],
                )

                # ---- t columns incl. sq: t_sb[p, 2r+h] = t[r, 128h+p]
                g_tmp = work.tile([128, 2, 4, DA], fp32, tag="g_tmp")
                nc.vector.tensor_mul(
                    g_tmp,
                    ctx_b.rearrange("p (r h) dd -> p h r dd", h=2),
                    qwc_aug[:, :]
                    .rearrange("p h (u dd) -> p h u dd", u=1)
                    .to_broadcast([128, 2, 4, DA]),
                )
                t_sb = small.tile([128, 8], fp32, tag="t_sb")
                nc.vector.tensor_reduce(
                    out=t_sb[:, :].rearrange("p (r h) -> p h r", h=2),
                    in_=g_tmp,
                    axis=X,
                    op=add,
                )

                # ---- CS tree (independent of softmax): csum[p,d]=sum_k ctx
                tmp4 = work.tile([128, 4, 128], fp32, tag="tmp4")
                nc.vector.tensor_add(
                    tmp4, ctx_b[:, 0:4, 0:128], ctx_b[:, 4:8, 0:128]
                )
                tmp2 = work.tile([128, 2, 128], fp32, tag="tmp2")
                nc.gpsimd.tensor_add(tmp2, tmp4[:, 0:2, :], tmp4[:, 2:4, :])
                csum = work.tile([128, 128], fp32, tag="csum")
                nc.gpsimd.tensor_add(csum, tmp2[:, 0, :], tmp2[:, 1, :])
                cs4_ps = ps_mm.tile([4, 128], fp32, tag="mm")
                nc.tensor.matmul(cs4_ps, cst["rsel"], csum, start=True, stop=True)
                cs4 = small.tile([4, 128], fp32, tag="cs4")
                nc.scalar.activation(
                    out=cs4, in_=cs4_ps, func=Copy, scale=1.0 / 256.0
                )

                # ---- softmax pieces, untransposed domain (|t| < ~8)
                e_sb = small.tile([128, 8], fp32, tag="e_sb")
                nc.scalar.activation(out=e_sb, in_=t_sb, func=Exp)
                ev = e_sb[:, :].rearrange("p (r h) -> p r h", r=4)
                # Z_r = sum_j e[r,j]: two accumulating column-sum matmuls
                pairs_ps = ps_sm.tile([4, 1], fp32, tag="sm")
                for h in range(2):
                    nc.tensor.matmul(
                        pairs_ps, ev[:, :, h], cst["ones1"],
                        start=(h == 0), stop=(h == 1),
                    )
                rec4 = small.tile([4, 1], fp32, tag="rec4")
                nc.vector.reciprocal(out=rec4, in_=pairs_ps)
                # U[j] = sum_r e[r,j] lives per-partition: strided reduce
                u_sb = small.tile([128, 2], fp32, tag="u_sb")
                nc.vector.tensor_reduce(
                    out=u_sb,
                    in_=e_sb[:, :].rearrange("p (r h) -> p h r", h=2),
                    axis=X,
                    op=add,
                )
                recu = small.tile([128, 2], fp32, tag="recu")
                nc.vector.reciprocal(out=recu, in_=u_sb)
                sqn = small.tile([128, 8], fp32, tag="sqn")
                nc.gpsimd.tensor_mul(
                    sqn[:, :].rearrange("p (r h) -> p r h", r=4),
                    ev,
                    recu[:, :]
                    .rearrange("p (u h) -> p u h", u=1)
                    .to_broadcast([128, 4, 2]),
                )
                sqnv = sqn[:, :].rearrange("p (r h) -> p r h", r=4)

                # ---- SM4raw[r',r] = sum_j sqn[r',j] e[r,j]
                sm4t_ps = ps_mm.tile([4, 4], fp32, tag="mm")
                for h in range(2):
                    nc.tensor.matmul(
                        sm4t_ps, sqnv[:, :, h], ev[:, :, h],
                        start=(h == 0), stop=(h == 1),
                    )
                sm4t = small.tile([4, 4], fp32, tag="sm4t")
                nc.scalar.copy(out=sm4t, in_=sm4t_ps)

                # ---- C2Q[r,d] = (1/Z_r) sum_j e[r,j] qry[j,d]
                c2q_ps = ps_mm.tile([4, 128], fp32, tag="mm")
                for h in range(2):
                    nc.tensor.matmul(
                        c2q_ps, ev[:, :, h], qry_b[:, h, :],
                        start=(h == 0), stop=(h == 1),
                    )
                c2q = small.tile([4, 128], fp32, tag="c2q")
                nc.scalar.activation(out=c2q, in_=c2q_ps, func=Copy, scale=rec4)
                repc_ps = ps_rep.tile([128, 128], fp32, tag="rep")
                nc.tensor.matmul(repc_ps, cst["b4"], c2q, start=True, stop=True)
                repc = small.tile([128, 128], fp32, tag="repc")
                nc.scalar.copy(out=repc, in_=repc_ps)

                # output stream b: broadcast C2Q columns (scalar queue)
                nc.scalar.dma_start(
                    out=out_d[b][:, 128:256].rearrange("(k p) c -> p k c", p=128),
                    in_=repc[:, :]
                    .rearrange("p (u d) -> p u d", u=1)
                    .to_broadcast([128, 8, 128]),
                )

                # ---- Q2C[r,d] = (1/(256 Z_r)) sum_r' SM4raw[r',r] CS[r',d]
                q2c_ps = ps_mm.tile([4, 128], fp32, tag="mm")
                nc.tensor.matmul(q2c_ps, sm4t, cs4, start=True, stop=True)
                q2c = small.tile([4, 128], fp32, tag="q2c")
                nc.scalar.activation(out=q2c, in_=q2c_ps, func=Copy, scale=rec4)
                repq_ps = ps_rep.tile([128, 128], fp32, tag="rep")
                nc.tensor.matmul(repq_ps, cst["b4"], q2c, start=True, stop=True)
                repq = small.tile([128, 128], fp32, tag="repq")
                nc.scalar.copy(out=repq, in_=repq_ps)

                # ---- product streams (sync queue), c first
                out_c = outp.tile([128, 8, 128], fp32, tag="out_c")
                nc.vector.tensor_mul(
                    out_c,
                    ctx_b[:, :, 0:128],
                    repc[:, :]
                    .rearrange("p (u d) -> p u d", u=1)
                    .to_broadcast([128, 8, 128]),
                )
                nc.sync.dma_start(
                    out=out_d[b][:, 256:384].rearrange("(k p) c -> p k c", p=128),
                    in_=out_c,
                )
                out_dd = outp.tile([128, 8, 128], fp32, tag="out_d")
                nc.gpsimd.tensor_mul(
                    out_dd,
                    ctx_b[:, :, 0:128],
                    repq[:, :]
                    .rearrange("p (u d) -> p u d", u=1)
                    .to_broadcast([128, 8, 128]),
                )
                nc.sync.dma_start(
                    out=out_d[b][:, 384:512].rearrange("(k p) c -> p k c", p=128),
                    in_=out_dd,
                )
    nc.compile()
    return nc


def _get_program():
    global _prog
    if _prog is None:
        _prog = _build_program()
    return _prog


def _make_const_inputs(w):
    w = np.ascontiguousarray(w, dtype=np.float32)
    w_q, w_c, w_m = w[:D, 0], w[D : 2 * D, 0], w[2 * D :, 0]
    p = np.arange(128)
    pairsel = (np.arange(8)[:, None] // 2 == np.arange(4)[None, :]).astype(
        np.float32
    )
    vals = {
        "wmb": np.broadcast_to(w_m[None, :], (128, 128)),
        "wcb": np.broadcast_to(w_c[None, :], (128, 128)),
        "wqb": np.broadcast_to(w_q[None, :], (128, 128)),
        "pairsel": pairsel,
        "b4": (np.arange(4)[:, None] == p[None, :] % 4).astype(np.float32),
        "ones1": np.ones((128, 1), np.float32),
        "rsel": (p[:, None] % 4 == np.arange(4)[None, :]).astype(np.float32),
    }
    packed = np.zeros((128, _CST_W), dtype=np.float32)
    for n, (parts, c0, cl) in _CST_COLS.items():
        packed[:parts, c0 : c0 + cl] = vals[n]
    return {"cstp": packed}


def _run(context, query, w, trace=False):
    from concourse.bass_utils import run_bass_kernel_spmd

    nc = _get_program()
    context = np.ascontiguousarray(context, dtype=np.float32)
    query = np.ascontiguousarray(query, dtype=np.float32)
    consts = _make_const_inputs(w)

    in_maps = []
    for c in range(NCORES):
        m = {
            "ctx": context[c * BPC : (c + 1) * BPC],
            "qry": query[c * BPC : (c + 1) * BPC],
        }
        m.update(consts)
        in_maps.append(m)

    res = run_bass_kernel_spmd(
        nc, in_maps, core_ids=list(range(NCORES)), trace=trace
    )
    out = np.concatenate([res.results[c]["out"] for c in range(NCORES)], axis=0)
    return out, res


def kernel(context, query, c_mask, q_mask, w):
    out, _ = _run(context, query, w, trace=False)
    return out


# revision 29
# speedup vs baseline: 1.1459x; 1.0822x over previous
"""Trainium2 Bass kernel for nn_ContextQueryAttentionLayer.

Math: with B,N,M,D = 32,1024,256,128 the reference's gather index collapses:
  idx[i,j] = (i*M + j) % N = 256*(i%4) + j          (since M=256, N=1024)
so the similarity matrix S (b,n,m) has only 4 distinct rows per batch,
S[b,i,:] = t[b, i%4, :] with t (4,256):
  t[r,j] = q_j.w_q + c_{256r+j}.w_c + sum_d q_{j,d} w_m_d c_{256r+j,d}
Both softmaxes, c2q, sm (a 4x4 matrix per batch) and q2c collapse to
rank-4-per-batch quantities, leaving a DMA-bound kernel:
  out[b,n] = [ctx_n, C2Q[n%4], ctx_n*C2Q[n%4], ctx_n*Q2C[n%4]]

v4: per-batch pipeline with the softmax in the untransposed [128, 8]
domain (t_sb[p, 2r+h] = t[r, 128h+p]; j%128 IS the partition index so
row sums / SM4 / C2Q contract directly over partitions on the PE).
Query prep is batched once (it only needs qry+consts). The q.w_q term
rides an augmented 129th column (ctx col 128 = 1, qwc col 128 = sq) so
one fused mul+reduce pass produces t directly. Normalizations ride the
PSUM->SBUF copies as per-partition activation scales; q2c contracts ctx
on the PE via 8 accumulating matmuls with M[p,r] = SM4raw[p%4,r].
Queues: sync = ctx loads + product streams; scalar = consts, qry,
raw-ctx stream, c2q-broadcast stream.
"""

import numpy as np

B, N, M, D = 32, 1024, 256, 128
NCORES = 8
BPC = B // NCORES  # batches per core
DA = D + 1  # augmented depth: col 128 carries the s_q term

_prog = None

# packed constant layout: name -> (partitions, col_start, col_len)
_CST_COLS = {
    "wmb": (128, 0, 128),
    "wcb": (128, 128, 128),
    "wqb": (128, 256, 128),
    "b4": (4, 384, 128),
    "pairsel": (8, 512, 4),
    "ones1": (128, 516, 1),
    "rsel": (128, 517, 4),
}
_CST_W = 524


def _build_program():
    import concourse.bacc as bacc
    import concourse.mybir as mybir
    from concourse.tile import TileContext

    fp32 = mybir.dt.float32
    nc = bacc.Bacc("TRN2", target_bir_lowering=False, name="cqattn")

    ctx_d = nc.dram_tensor("ctx", [BPC, N, D], fp32, kind="ExternalInput")
    qry_d = nc.dram_tensor("qry", [BPC, M, D], fp32, kind="ExternalInput")
    cstp_d = nc.dram_tensor("cstp", [128, _CST_W], fp32, kind="ExternalInput")
    out_d = nc.dram_tensor("out", [BPC, N, 4 * D], fp32, kind="ExternalOutput")

    Exp = mybir.ActivationFunctionType.Exp
    Copy = mybir.ActivationFunctionType.Copy
    add = mybir.AluOpType.add
    X = mybir.AxisListType.X

    with TileContext(nc) as tc:
        with (
            tc.tile_pool(name="consts", bufs=1) as consts,
            tc.tile_pool(name="io", bufs=1) as io,
            tc.tile_pool(name="work", bufs=2) as work,
            tc.tile_pool(name="small", bufs=2) as small,
            tc.tile_pool(name="outp", bufs=2) as outp,
            tc.tile_pool(name="ps_sm", bufs=2, space="PSUM") as ps_sm,
            tc.tile_pool(name="ps_mm", bufs=2, space="PSUM") as ps_mm,
            tc.tile_pool(name="ps_rep", bufs=2, space="PSUM") as ps_rep,
        ):
            # ---- consts then qry on the scalar (ACT) queue, first thing
            cstp = consts.tile([128, _CST_W], fp32, tag="cstp", name="cstp")
            nc.scalar.dma_start(out=cstp, in_=cstp_d[...])
            cst = {
                n: cstp[:p, c0 : c0 + cl] for n, (p, c0, cl) in _CST_COLS.items()
            }
            qry_mega = io.tile([128, BPC, 2, 128], fp32, tag="qry", name="qry_mega")
            nc.scalar.dma_start(
                out=qry_mega,
                in_=qry_d.rearrange("b (h p) d -> p b h d", p=128),
            )

            # ---- ctx loads back-to-back on the sync queue (col 128 = 1.0)
            ctx_mega = io.tile([128, BPC, 8, DA], fp32, tag="ctx", name="ctx_mega")
            nc.vector.memset(ctx_mega[:, :, :, 128:129], 1.0)
            for b in range(BPC):
                nc.sync.dma_start(
                    out=ctx_mega[:, b, :, 0:128],
                    in_=ctx_d[b].rearrange("(k p) d -> p k d", p=128),
                )

            # ---- batched query prep: qwc_aug[p,b,h,:128] = qry*wm + wc,
            #      qwc_aug[p,b,h,128] = sq = qry . wq
            qwc_aug = work.tile([128, BPC, 2, DA], fp32, tag="qwc", name="qwc")
            qa = qwc_aug[:, :, :, 0:128]
            nc.vector.tensor_mul(
                qa,
                qry_mega,
                cst["wmb"]
                .rearrange("p (u v d) -> p u v d", u=1, v=1)
                .to_broadcast([128, BPC, 2, 128]),
            )
            nc.vector.tensor_add(
                qa,
                qa,
                cst["wcb"]
                .rearrange("p (u v d) -> p u v d", u=1, v=1)
                .to_broadcast([128, BPC, 2, 128]),
            )
            sq_tmp = work.tile([128, BPC, 2, 128], fp32, tag="sq_tmp")
            nc.vector.tensor_mul(
                sq_tmp,
                qry_mega,
                cst["wqb"]
                .rearrange("p (u v d) -> p u v d", u=1, v=1)
                .to_broadcast([128, BPC, 2, 128]),
            )
            nc.vector.tensor_reduce(
                out=qwc_aug[:, :, :, 128:129].rearrange("p b h o -> p b (h o)"),
                in_=sq_tmp,
                axis=X,
                op=add,
            )

            for b in range(BPC):
                ctx_b = ctx_mega[:, b]
                qry_b = qry_mega[:, b]

                # output stream a: raw context columns (scalar queue)
                nc.scalar.dma_start(
                    out=out_d[b][:, 0:128].rearrange("(k p) c -> p k c", p=128),
                    in_=ctx_b[:, :, 0:128],
                )

                # ---- t columns incl. sq: t_sb[p, 2r+h] = t[r, 128h+p]
                g_tmp = work.tile([128, 2, 4, DA], fp32, tag="g_tmp")
                nc.vector.tensor_mul(
                    g_tmp,
                    ctx_b.rearrange("p (r h) dd -> p h r dd", h=2),
                    qwc_aug[:, b]
                    .rearrange("p h (u dd) -> p h u dd", u=1)
                    .to_broadcast([128, 2, 4, DA]),
                )
                t_sb = small.tile([128, 8], fp32, tag="t_sb")
                nc.vector.tensor_reduce(
                    out=t_sb[:, :].rearrange("p (r h) -> p h r", h=2),
                    in_=g_tmp,
                    axis=X,
                    op=add,
                )

                # ---- CS tree (independent of softmax): csum[p,d]=sum_k ctx
                tmp4 = work.tile([128, 4, 128], fp32, tag="tmp4")
                nc.vector.tensor_add(
                    tmp4, ctx_b[:, 0:4, 0:128], ctx_b[:, 4:8, 0:128]
                )
                tmp2 = work.tile([128, 2, 128], fp32, tag="tmp2")
                nc.gpsimd.tensor_add(tmp2, tmp4[:, 0:2, :], tmp4[:, 2:4, :])
                csum = work.tile([128, 128], fp32, tag="csum")
                nc.gpsimd.tensor_add(csum, tmp2[:, 0, :], tmp2[:, 1, :])
                cs4_ps = ps_mm.tile([4, 128], fp32, tag="mm")
                nc.tensor.matmul(cs4_ps, cst["rsel"], csum, start=True, stop=True)
                cs4 = small.tile([4, 128], fp32, tag="cs4")
                nc.scalar.activation(
                    out=cs4, in_=cs4_ps, func=Copy, scale=1.0 / 256.0
                )

                # ---- softmax pieces, untransposed domain (|t| < ~8)
                e_sb = small.tile([128, 8], fp32, tag="e_sb")
                nc.scalar.activation(out=e_sb, in_=t_sb, func=Exp)
                ev = e_sb[:, :].rearrange("p (r h) -> p r h", r=4)
                # Z_r = sum_j e[r,j]: two accumulating column-sum matmuls
                pairs_ps = ps_sm.tile([4, 1], fp32, tag="sm")
                for h in range(2):
                    nc.tensor.matmul(
                        pairs_ps, ev[:, :, h], cst["ones1"],
                        start=(h == 0), stop=(h == 1),
                    )
                rec4 = small.tile([4, 1], fp32, tag="rec4")
                nc.vector.reciprocal(out=rec4, in_=pairs_ps)
                # U[j] = sum_r e[r,j] lives per-partition: strided reduce
                u_sb = small.tile([128, 2], fp32, tag="u_sb")
                nc.vector.tensor_reduce(
                    out=u_sb,
                    in_=e_sb[:, :].rearrange("p (r h) -> p h r", h=2),
                    axis=X,
                    op=add,
                )
                recu = small.tile([128, 2], fp32, tag="recu")
                nc.vector.reciprocal(out=recu, in_=u_sb)
                sqn = small.tile([128, 8], fp32, tag="sqn")
                nc.gpsimd.tensor_mul(
                    sqn[:, :].rearrange("p (r h) -> p r h", r=4),
                    ev,
                    recu[:, :]
                    .rearrange("p (u h) -> p u h", u=1)
                    .to_broadcast([128, 4, 2]),
                )
                sqnv = sqn[:, :].rearrange("p (r h) -> p r h", r=4)

                # ---- SM4raw[r',r] = sum_j sqn[r',j] e[r,j]
                sm4t_ps = ps_mm.tile([4, 4], fp32, tag="mm")
                for h in range(2):
                    nc.tensor.matmul(
                        sm4t_ps, sqnv[:, :, h], ev[:, :, h],
                        start=(h == 0), stop=(h == 1),
                    )
                sm4t = small.tile([4, 4], fp32, tag="sm4t")
                nc.scalar.copy(out=sm4t, in_=sm4t_ps)

                # ---- C2Q[r,d] = (1/Z_r) sum_j e[r,j] qry[j,d]
                c2q_ps = ps_mm.tile([4, 128], fp32, tag="mm")
                for h in range(2):
                    nc.tensor.matmul(
                        c2q_ps, ev[:, :, h], qry_b[:, h, :],
                        start=(h == 0), stop=(h == 1),
                    )
                c2q = small.tile([4, 128], fp32, tag="c2q")
                nc.scalar.activation(out=c2q, in_=c2q_ps, func=Copy, scale=rec4)
                repc_ps = ps_rep.tile([128, 128], fp32, tag="rep")
                nc.tensor.matmul(repc_ps, cst["b4"], c2q, start=True, stop=True)
                repc = small.tile([128, 128], fp32, tag="repc")
                nc.scalar.copy(out=repc, in_=repc_ps)

                # output stream b: broadcast C2Q columns (scalar queue)
                nc.scalar.dma_start(
                    out=out_d[b][:, 128:256].rearrange("(k p) c -> p k c", p=128),
                    in_=repc[:, :]
                    .rearrange("p (u d) -> p u d", u=1)
                    .to_broadcast([128, 8, 128]),
                )

                # ---- Q2C[r,d] = (1/(256 Z_r)) sum_r' SM4raw[r',r] CS[r',d]
                q2c_ps = ps_mm.tile([4, 128], fp32, tag="mm")
                nc.tensor.matmul(q2c_ps, sm4t, cs4, start=True, stop=True)
                q2c = small.tile([4, 128], fp32, tag="q2c")
                nc.scalar.activation(out=q2c, in_=q2c_ps, func=Copy, scale=rec4)
                repq_ps = ps_rep.tile([128, 128], fp32, tag="rep")
                nc.tensor.matmul(repq_ps, cst["b4"], q2c, start=True, stop=True)
                repq = small.tile([128, 128], fp32, tag="repq")
                nc.scalar.copy(out=repq, in_=repq_ps)

                # ---- product streams (sync queue), c first
                out_c = outp.tile([128, 8, 128], fp32, tag="out_c")
                nc.vector.tensor_mul(
                    out_c,
                    ctx_b[:, :, 0:128],
                    repc[:, :]
                    .rearrange("p (u d) -> p u d", u=1)
                    .to_broadcast([128, 8, 128]),
                )
                nc.sync.dma_start(
                    out=out_d[b][:, 256:384].rearrange("(k p) c -> p k c", p=128),
                    in_=out_c,
                )
                out_dd = outp.tile([128, 8, 128], fp32, tag="out_d")
                nc.gpsimd.tensor_mul(
                    out_dd,
                    ctx_b[:, :, 0:128],
                    repq[:, :]
                    .rearrange("p (u d) -> p u d", u=1)
                    .to_broadcast([128, 8, 128]),
                )
                nc.sync.dma_start(
                    out=out_d[b][:, 384:512].rearrange("(k p) c -> p k c", p=128),
                    in_=out_dd,
                )
    nc.compile()
    return nc


def _get_program():
    global _prog
    if _prog is None:
        _prog = _build_program()
    return _prog


def _make_const_inputs(w):
    w = np.ascontiguousarray(w, dtype=np.float32)
    w_q, w_c, w_m = w[:D, 0], w[D : 2 * D, 0], w[2 * D :, 0]
    p = np.arange(128)
    pairsel = (np.arange(8)[:, None] // 2 == np.arange(4)[None, :]).astype(
        np.float32
    )
    vals = {
        "wmb": np.broadcast_to(w_m[None, :], (128, 128)),
        "wcb": np.broadcast_to(w_c[None, :], (128, 128)),
        "wqb": np.broadcast_to(w_q[None, :], (128, 128)),
        "pairsel": pairsel,
        "b4": (np.arange(4)[:, None] == p[None, :] % 4).astype(np.float32),
        "ones1": np.ones((128, 1), np.float32),
        "rsel": (p[:, None] % 4 == np.arange(4)[None, :]).astype(np.float32),
    }
    packed = np.zeros((128, _CST_W), dtype=np.float32)
    for n, (parts, c0, cl) in _CST_COLS.items():
        packed[:parts, c0 : c0 + cl] = vals[n]
    return {"cstp": packed}


def _run(context, query, w, trace=False):
    from concourse.bass_utils import run_bass_kernel_spmd

    nc = _get_program()
    context = np.ascontiguousarray(context, dtype=np.float32)
    query = np.ascontiguousarray(query, dtype=np.float32)
    consts = _make_const_inputs(w)

    in_maps = []
    for c in range(NCORES):
        m = {
            "ctx": context[c * BPC : (c + 1) * BPC],
            "qry": query[c * BPC : (c + 1) * BPC],
        }
        m.update(consts)
        in_maps.append(m)

    res = run_bass_kernel_spmd(
        nc, in_maps, core_ids=list(range(NCORES)), trace=trace
    )
    out = np.concatenate([res.results[c]["out"] for c in range(NCORES)], axis=0)
    return out, res


def kernel(context, query, c_mask, q_mask, w):
    out, _ = _run(context, query, w, trace=False)
    return out
